# revision 12
# baseline (speedup 1.0000x reference)
"""Trainium2 Bass kernel for nn_LinearReg_55508157333593.

loss = (c_omega * 0.001 / N) * sum of L2 norms of all 25-float groups of
weight [100000, 800] f32.  Flat buffer = 3.2M consecutive 25-float groups,
sharded across 8 NeuronCores (10M floats each, [128, 78125] per core).

v3 pipeline (raw Bass), drain-optimized:
  SP   streams chunks (12.8KB/partition descriptors keep the DMA at peak;
       small chunks only where the endgame needs them)
  ACT  squares each chunk in place (f32) or into bf16 z-buffers (tail tree
       chunks), plus mid-stream sqrt+accum segments over gs -> pr columns
  DVE  group sums: f32 3D-AP tensor_reduce for bulk chunks; for the tail
       chunks a bf16 pairwise fold tree over host-prepared k-major slabs
       (25 slabs of G) running at the DVE 2x packed rate, sem-chained and
       interleaved across chunks so nothing needs a pipeline drain.
Endgame: the two small R chunks are streamed EARLY (their compute hides
mid-stream); the tail is a descending run of tree chunks; the last 25
floats are squared+summed by a single ACT activation with accum_out.
Late group sums ship raw (pr columns); the host sqrts those and does the
final f64 summation. Output per core: pr [128, ncol].
"""

import sys

import numpy as np

if "/opt/trn_rl_repo" not in sys.path:
    sys.path.insert(0, "/opt/trn_rl_repo")

N_CORES = 8
P = 128
GROUP = 25
C_OMEGA = 0.001
N_ROWS = 100000
ROW = 800
F_PER_PART = (N_ROWS * ROW) // (N_CORES * P)   # 78125

# stream-ordered chunk specs: (size, kind)
#  B = bulk f32 square+reduce -> gs (device sqrt via segs)
#  Rb = f32 square+reduce -> pr raw cols
#  T = k-major bf16 fold tree -> pr raw cols
#  A = ACT square+accum_out -> pr raw col (must be 25 floats, last)
CHUNKS = (
    [(3200, "B")] * 21
    + [(500, "R"), (400, "R")]
    + [(3200, "R")]            # 22nd big chunk: reduce -> raw cols
    + [(1600, "T"), (1600, "T"), (1600, "T"), (1200, "T"), (800, "T")]
    + [(25, "A")]
)
# sqrt segments over gs (bulk = B chunks only): (end_B_chunk_exclusive,
# emit_after_chunk_index)  -- indices into CHUNKS for the emit point
SEGS = [(5, 6), (9, 10), (13, 14), (17, 18), (21, 23)]
IN_BUFS = 10

_compiled = None
LAST_RESULTS = None


def build(chunks=None, segs=None, in_bufs=IN_BUFS):
    from concourse import bacc, mybir
    from concourse.alu_op_type import AluOpType

    if chunks is None:
        chunks, segs = CHUNKS, SEGS
    n = len(chunks)
    sizes = [c[0] for c in chunks]
    kinds = [c[1] for c in chunks]
    f_per_part = sum(sizes)
    assert all(s % GROUP == 0 for s in sizes)
    assert kinds[-1] == "A" and sizes[-1] == GROUP
    offs = [sum(sizes[:i]) for i in range(n)]
    gpcs = [s // GROUP for s in sizes]
    for i, k in enumerate(kinds):
        if k == "T":
            assert gpcs[i] % 2 == 0, "tree chunks need even group count"

    n_segs = len(segs)
    bulk_ids = [i for i, k in enumerate(kinds) if k == "B"]
    goffs = {}
    acc = 0
    for i in bulk_ids:
        goffs[i] = acc
        acc += gpcs[i]
    bulk_g = acc
    roffs = {}
    racc = n_segs
    for i in range(n):
        if kinds[i] != "B":
            roffs[i] = racc
            racc += gpcs[i]
    ncol = racc
    # seg group ranges: segs are over the bulk chunks in order
    seg_gr = []
    prev_b = 0
    for b, _ in segs:
        lo = goffs[bulk_ids[prev_b]] if prev_b < len(bulk_ids) else bulk_g
        hi = (goffs[bulk_ids[b]] if b < len(bulk_ids) else bulk_g)
        seg_gr.append((lo, hi))
        prev_b = b
    assert segs[-1][0] == len(bulk_ids)
    emit_after = {e: si for si, (_, e) in enumerate(segs)}
    # red_sem count needed before seg si: number of chunks (in stream order)
    # up to and including the last bulk chunk it covers
    seg_gate = {}
    for si, (b, _) in enumerate(segs):
        last_bulk = bulk_ids[b - 1]
        seg_gate[si] = last_bulk + 1

    f32 = mybir.dt.float32
    bf16 = mybir.dt.bfloat16
    Act = mybir.ActivationFunctionType
    max_sz = max(sizes)

    nc = bacc.Bacc("TRN2", target_bir_lowering=False, debug=False,
                   num_devices=N_CORES)
    x = nc.dram_tensor("x", [P, f_per_part], f32, kind="ExternalInput").ap()
    out = nc.dram_tensor("out", [P, ncol], f32, kind="ExternalOutput").ap()

    B = in_bufs
    ring = nc.alloc_sbuf_tensor("ring", [P, B * max_sz], f32).ap()
    t_g = sum(sizes[i] for i in range(n) if kinds[i] == "T")
    zr = nc.alloc_sbuf_tensor("zr", [P, max(t_g, 1)], bf16).ap()
    zoffs = {}
    za = 0
    for i in range(n):
        if kinds[i] == "T":
            zoffs[i] = za
            za += sizes[i]
    gs = nc.alloc_sbuf_tensor("gs", [P, bulk_g], f32).ap()
    pr = nc.alloc_sbuf_tensor("pr", [P, ncol], f32).ap()
    dm = nc.alloc_sbuf_tensor("dm_scratch", [1, 1], f32).ap()
    ones = nc.const_aps.aps[(f32, 1.0)]

    tslot = [ring[:, b * max_sz:(b + 1) * max_sz] for b in range(B)]

    dma_sems = [nc.alloc_semaphore(f"dma_sem{b}") for b in range(B)]
    out_sem = nc.alloc_semaphore("out_sem")
    sq_sem = nc.alloc_semaphore("sq_sem")       # +1 per ACT square, chunk order
    red_sem = nc.alloc_semaphore("red_sem")     # +1 per chunk (sums written)
    tree_sem = nc.alloc_semaphore("tree_sem")   # +1 per DVE tree instruction
    sqrt_sem = nc.alloc_semaphore("sqrt_sem")   # +1 per seg accum readout

    def emit_sp(sp):
        for i in range(n):
            if i >= B:
                sp.wait_ge(red_sem, i - B + 1)
            sp.dma_start(
                tslot[i % B][:, :sizes[i]],
                x[:, offs[i]:offs[i] + sizes[i]],
            ).then_inc(dma_sems[i % B], 16)
        sp.wait_ge(red_sem, n)
        sp.wait_ge(sqrt_sem, n_segs)
        sp.dma_start(out, pr).then_inc(out_sem, 16)
        sp.wait_ge(out_sem, 16)

    def emit_act(act):
        act.activation(dm, ones[0:1, :], Act.Sqrt)

        def emit_seg(si):
            glo, ghi = seg_gr[si]
            act.wait_ge(red_sem, seg_gate[si])
            act.activation(gs[:, glo:ghi], gs[:, glo:ghi], Act.Sqrt,
                           accum_out=pr[:, si:si + 1]).then_inc(sqrt_sem, 1)

        for i in range(n):
            act.wait_ge(dma_sems[i % B], 16 * (i // B + 1))
            s = sizes[i]
            if kinds[i] == "A":
                act.activation(tslot[i % B][:, :s], tslot[i % B][:, :s],
                               Act.Square,
                               accum_out=pr[:, roffs[i]:roffs[i] + 1])\
                    .then_inc(red_sem, 1)
            elif kinds[i] == "T":
                act.activation(zr[:, zoffs[i]:zoffs[i] + s],
                               tslot[i % B][:, :s],
                               Act.Square).then_inc(sq_sem, 1)
            else:
                act.activation(tslot[i % B][:, :s], tslot[i % B][:, :s],
                               Act.Square).then_inc(sq_sem, 1)
            if i in emit_after:
                emit_seg(emit_after[i])

    def emit_dve(dve):
        # tree instructions are sem-chained (tree_sem); dependent levels of
        # the same chunk wait on the chunk's previous level. Chunks are
        # emitted in arrival order; adjacent T chunks naturally interleave
        # because each level's wait is usually already satisfied.
        tcount = 0
        sq_count = 0

        def tree(i):
            nonlocal tcount
            s = sizes[i]
            g = gpcs[i]
            zz = zr[:, zoffs[i]:zoffs[i] + s]
            dst = pr[:, roffs[i]:roffs[i] + g]
            lv = [
                (zz[:, 23 * g:24 * g], zz[:, 23 * g:24 * g], zz[:, 24 * g:25 * g]),
                (zz[:, 0:12 * g], zz[:, 0:12 * g], zz[:, 12 * g:24 * g]),
                (zz[:, 0:6 * g], zz[:, 0:6 * g], zz[:, 6 * g:12 * g]),
                (zz[:, 0:3 * g], zz[:, 0:3 * g], zz[:, 3 * g:6 * g]),
                (zz[:, 0:g], zz[:, 0:g], zz[:, g:2 * g]),
                (dst, zz[:, 0:g], zz[:, 2 * g:3 * g]),
            ]
            for li, (o, a, b2) in enumerate(lv):
                if li == 0:
                    dve.wait_ge(sq_sem, sq_count)
                else:
                    dve.wait_ge(tree_sem, tcount)  # own prev level done
                ins = dve.tensor_tensor(o, a, b2, op=AluOpType.add)
                if li == len(lv) - 1:
                    ins.then_inc(red_sem, 1)
                else:
                    ins.then_inc(tree_sem, 1)
                    tcount += 1

        for i in range(n):
            s = sizes[i]
            if kinds[i] == "A":
                continue    # ACT handles it end-to-end
            sq_count += 1
            if kinds[i] == "T":
                tree(i)
                continue
            g = gpcs[i]
            if kinds[i] == "B":
                dst = gs[:, goffs[i]:goffs[i] + g]
            else:
                dst = pr[:, roffs[i]:roffs[i] + g]
            dve.wait_ge(sq_sem, sq_count)
            dve.tensor_reduce(
                dst,
                tslot[i % B][:, :s].rearrange("p (g k) -> p g k", k=GROUP),
                axis=mybir.AxisListType.X, op=AluOpType.add,
            ).then_inc(red_sem, 1)

    emit_sp(nc.sync)
    emit_act(nc.scalar)
    emit_dve(nc.vector)

    nc.compile()
    meta = dict(chunks=chunks, segs=segs, n_segs=n_segs, ncol=ncol,
                f_per_part=f_per_part)
    return nc, meta


def _relayout(flat2d, chunks):
    """Copy with T chunks k-major per row: blk[g,25] -> blk[25,g]."""
    out = flat2d.copy()
    off = 0
    for s, kind in chunks:
        if kind == "T":
            g = s // GROUP
            blk = out[:, off:off + s].reshape(-1, g, GROUP)
            out[:, off:off + s] = np.ascontiguousarray(
                blk.transpose(0, 2, 1)).reshape(-1, s)
        off += s
    return out


def kernel(weight, c_omega):
    global _compiled, LAST_RESULTS
    from concourse.bass_utils import run_bass_kernel_spmd

    if _compiled is None:
        _compiled = build()
    nc, meta = _compiled

    w = np.asarray(weight)
    if w.dtype != np.float32:
        w = w.astype(np.float32)
    flat = np.ascontiguousarray(w).reshape(N_CORES * P, F_PER_PART)
    xk = _relayout(flat, meta["chunks"])
    in_maps = [{"x": xk[c * P:(c + 1) * P]} for c in range(N_CORES)]
    LAST_RESULTS = run_bass_kernel_spmd(nc, in_maps,
                                        core_ids=list(range(N_CORES)))
    S = meta["n_segs"]
    total = 0.0
    for r in LAST_RESULTS.results:
        o = r["out"].astype(np.float64)
        total += o[:, :S].sum() + np.sqrt(o[:, S:]).sum()
    loss = total / N_ROWS * (C_OMEGA * float(c_omega))
    return np.float32(loss)


def selftest_sim():
    """CoreSim check on a scaled-down instance; returns rel err."""
    from concourse.bass_interp import CoreSim

    chunks = [(250, "B"), (75, "R"), (250, "B"), (200, "T"), (100, "T"),
              (25, "A")]
    segs = [(1, 2), (2, 3)]
    f = sum(c[0] for c in chunks)
    nc, meta = build(chunks=chunks, segs=segs, in_bufs=3)
    rng = np.random.default_rng(0)
    xv = rng.standard_normal((P, f)).astype(np.float32)
    xk = _relayout(xv, chunks)
    sim = CoreSim(nc)
    sim.tensor("x")[:] = xk
    sim.simulate()
    o = np.array(sim.tensor("out")).astype(np.float64)
    S = meta["n_segs"]
    got = o[:, :S].sum() + np.sqrt(o[:, S:]).sum()
    g = xv.reshape(P, f // GROUP, GROUP).astype(np.float64)
    want = np.sqrt((g ** 2).sum(-1)).sum()
    return abs(got - want) / abs(want)


# revision 13
# speedup vs baseline: 1.0017x; 1.0017x over previous
"""Trainium2 Bass kernel for nn_LinearReg_55508157333593.

Computes: loss = (c_omega * 0.001 / N) * sum over all rows/groups of
L2 norms of 25-element groups of weight [100000, 800] f32.

The whole buffer is 3.2M consecutive 25-float groups, sharded across 8
NeuronCores (10M floats each) streamed through SBUF as [128, 78125].

Raw-Bass pipeline (identical DMA structure to the proven v1: 24 x 3125
chunks + descending tail, 12-slot ring), with a drain-optimized endgame:
  SP:  DMA chunk i into slot i%B (per-slot completion sems)
  ACT: square in place; the last big chunks are split into 3 compute
       pieces so their reduces pipeline instead of trailing monolithically
  DVE: per-group (25) reduce; bulk chunks -> gs, late chunks -> pr columns
       (raw group sums, host applies sqrt)
  ACT: sqrt+accum_out segments over gs -> pr columns; two segments run
       mid-stream, the last one after the final square (gated only by a
       mid-stream reduce, so it never blocks tail squares)
Output: pr [128, 3 + 234] per core; host sqrts raw columns and sums in
f64 across cores (as v1 already did for its scalar outputs).
"""

import sys

import numpy as np

if "/opt/trn_rl_repo" not in sys.path:
    sys.path.insert(0, "/opt/trn_rl_repo")

N_CORES = 8
P = 128
GROUP = 25
C_OMEGA = 0.001
N_ROWS = 100000
ROW = 800
F_PER_PART = (N_ROWS * ROW) // (N_CORES * P)   # 78125

SCHEDULE = [3125] * 24 + [625] * 4 + [500, 125]
# compute-piece splits per chunk (chunks 0-1 for DVE wakeup, 21-23 so the
# last big reduces pipeline with their squares)
SPLITS = {0: 2, 1: 2, 21: 3, 22: 3, 23: 3}
RAW_FROM = 23            # chunks >= this ship raw group sums
# sqrt segs over bulk chunks [0, RAW_FROM): (end_chunk_exclusive,
# emit_after_sq_of_chunk or None = after all squares)
SEGS = [(8, 10), (16, 18), (23, None)]

_compiled = None
LAST_RESULTS = None


def build(f_per_part=F_PER_PART, schedule=None, in_bufs=12, splits=None,
          segs=None, raw_from=None):
    """Build and compile the per-core raw-Bass program."""
    from concourse import bacc, mybir

    if schedule is None:
        schedule = SCHEDULE
        splits = SPLITS
        segs = SEGS
        raw_from = RAW_FROM
    if splits is None:
        splits = {}
    n = len(schedule)
    assert sum(schedule) == f_per_part
    assert all(s % GROUP == 0 for s in schedule)
    offs = [sum(schedule[:i]) for i in range(n)]
    gpcs = [s // GROUP for s in schedule]
    n_segs = len(segs)

    # pieces: (chunk, lo, hi) float ranges within the chunk
    pieces = []
    for i in range(n):
        s = schedule[i]
        k = splits.get(i, 1)
        if k > 1 and s >= k * GROUP:
            per = (s // k // GROUP) * GROUP
            lo = 0
            for j in range(k - 1):
                pieces.append((i, lo, lo + per))
                lo += per
            pieces.append((i, lo, s))
        else:
            pieces.append((i, 0, s))
    last_piece = {}
    for p, (c, _, _) in enumerate(pieces):
        last_piece[c] = p
    r_of = last_piece            # reduce ops mirror pieces 1:1
    n_pieces = len(pieces)

    # group-column layout: bulk chunks -> gs, raw chunks -> pr columns
    goffs = {}
    acc = 0
    for i in range(raw_from):
        goffs[i] = acc
        acc += gpcs[i]
    bulk_g = acc
    roffs = {}
    racc = n_segs
    for i in range(raw_from, n):
        roffs[i] = racc
        racc += gpcs[i]
    ncol = racc
    assert segs[-1][0] == raw_from

    seg_gr = []
    prev = 0
    for b, _ in segs:
        seg_gr.append((goffs[prev], goffs[b] if b < raw_from else bulk_g))
        prev = b
    emit_after = {}
    for si, (_, e) in enumerate(segs):
        if e is not None:
            emit_after[e] = si

    max_sz = max(schedule)
    f32 = mybir.dt.float32
    Act = mybir.ActivationFunctionType

    nc = bacc.Bacc("TRN2", target_bir_lowering=False, debug=False,
                   num_devices=N_CORES)
    x = nc.dram_tensor("x", [P, f_per_part], f32, kind="ExternalInput").ap()
    out = nc.dram_tensor("out", [1 if False else P, ncol], f32,
                         kind="ExternalOutput").ap()

    B = in_bufs
    ring = nc.alloc_sbuf_tensor("ring", [P, B * max_sz], f32).ap()
    t = [ring[:, b * max_sz:(b + 1) * max_sz] for b in range(B)]

    gs_all = nc.alloc_sbuf_tensor("gs_all", [P, bulk_g], f32).ap()
    pr = nc.alloc_sbuf_tensor("pr", [P, ncol], f32).ap()
    dm = nc.alloc_sbuf_tensor("dm_scratch", [1, 1], f32).ap()
    ones = nc.const_aps.aps[(f32, 1.0)]

    dma_sems = [nc.alloc_semaphore(f"dma_sem{b}") for b in range(B)]
    out_sem = nc.alloc_semaphore("out_sem")
    sq_sem = nc.alloc_semaphore("sq_sem")       # ACT square piece done
    red_sem = nc.alloc_semaphore("red_sem")     # DVE reduce piece done
    sqrt_sem = nc.alloc_semaphore("sqrt_sem")   # ACT seg accum readouts

    def emit_sp(sp):
        for i in range(n):
            if i >= B:
                sp.wait_ge(red_sem, r_of[i - B] + 1)
            sp.dma_start(
                t[i % B][:, :schedule[i]], x[:, offs[i]:offs[i] + schedule[i]]
            ).then_inc(dma_sems[i % B], 16)
        sp.wait_ge(red_sem, n_pieces)
        sp.wait_ge(sqrt_sem, n_segs)
        sp.dma_start(out, pr).then_inc(out_sem, 16)
        sp.wait_ge(out_sem, 16)

    def emit_act(act):
        # table prefetch: first activation is a Sqrt so the one table set
        # (sqrt_and_others, which also contains square) serves everything
        act.activation(dm, ones[0:1, :], Act.Sqrt)

        def emit_seg(si):
            b, _ = segs[si]
            glo, ghi = seg_gr[si]
            act.wait_ge(red_sem, r_of[b - 1] + 1)
            act.activation(gs_all[:, glo:ghi], gs_all[:, glo:ghi], Act.Sqrt,
                           accum_out=pr[:, si:si + 1]).then_inc(sqrt_sem, 1)

        prev_chunk = -1
        for c, lo, hi in pieces:
            if c != prev_chunk:
                act.wait_ge(dma_sems[c % B], 16 * (c // B + 1))
                prev_chunk = c
            act.activation(t[c % B][:, lo:hi], t[c % B][:, lo:hi],
                           Act.Square).then_inc(sq_sem, 1)
            if hi == schedule[c] and c in emit_after:
                emit_seg(emit_after[c])
        for si, (_, e) in enumerate(segs):
            if e is None:
                emit_seg(si)

    def emit_dve(dve):
        for p, (c, lo, hi) in enumerate(pieces):
            dve.wait_ge(sq_sem, p + 1)
            base = (c % B) * max_sz
            if c < raw_from:
                dst = gs_all[:, goffs[c] + lo // GROUP:goffs[c] + hi // GROUP]
            else:
                dst = pr[:, roffs[c] + lo // GROUP:roffs[c] + hi // GROUP]
            dve.reduce_sum(
                dst,
                ring[:, base + lo:base + hi].rearrange("p (g k) -> p g k",
                                                       k=GROUP),
                axis=mybir.AxisListType.X,
            ).then_inc(red_sem, 1)

    emit_sp(nc.sync)
    emit_act(nc.scalar)
    emit_dve(nc.vector)

    nc.compile()
    meta = dict(n_segs=n_segs, ncol=ncol)
    return nc, meta


def kernel(weight, c_omega):
    global _compiled, LAST_RESULTS
    from concourse.bass_utils import run_bass_kernel_spmd

    if _compiled is None:
        _compiled = build()
    nc, meta = _compiled

    w = np.asarray(weight)
    if w.dtype != np.float32:
        w = w.astype(np.float32)
    w = np.ascontiguousarray(w)
    flat = w.reshape(-1)
    per_core = flat.size // N_CORES
    in_maps = [
        {"x": flat[c * per_core:(c + 1) * per_core].reshape(P, F_PER_PART)}
        for c in range(N_CORES)
    ]
    LAST_RESULTS = run_bass_kernel_spmd(nc, in_maps,
                                        core_ids=list(range(N_CORES)))
    S = meta["n_segs"]
    total = 0.0
    for r in LAST_RESULTS.results:
        o = r["out"].astype(np.float64)
        total += o[:, :S].sum() + np.sqrt(o[:, S:]).sum()
    loss = total / N_ROWS * (C_OMEGA * float(c_omega))
    return np.float32(loss)


def selftest_sim():
    """CoreSim check on a scaled-down instance; returns max rel err."""
    from concourse.bass_interp import CoreSim

    schedule = [250, 250, 150, 75, 50]
    splits = {0: 2, 2: 3}
    segs = [(1, 2), (3, None)]
    raw_from = 3
    f = sum(schedule)
    nc, meta = build(f_per_part=f, schedule=schedule, in_bufs=3,
                     splits=splits, segs=segs, raw_from=raw_from)
    rng = np.random.default_rng(0)
    xv = rng.standard_normal((P, f)).astype(np.float32)
    sim = CoreSim(nc)
    sim.tensor("x")[:] = xv
    sim.simulate()
    o = np.array(sim.tensor("out")).astype(np.float64)
    S = meta["n_segs"]
    got = o[:, :S].sum() + np.sqrt(o[:, S:]).sum()
    g = xv.reshape(P, f // GROUP, GROUP).astype(np.float64)
    want = np.sqrt((g ** 2).sum(-1)).sum()
    return abs(got - want) / abs(want)
